# revision 12
# baseline (speedup 1.0000x reference)
"""Multi-head attention (B=2, S=2048, H=1024, 16 heads x 64d) on 8 trn2 cores.

Sharding: tensor-parallel over heads (2 heads/core). Each core computes the
qkv projection for its 384 output features, attention for its 2 heads, and a
partial o_proj ([4096,1024] over its 128-feature slice). Host sums the 8
fp16 partials and adds b_o.

v2 layout/scheduling notes (vs the 306us baseline):
  - attention runs as (b, h, qc-pair) passes: two 512-col q-chunks (A/B)
    interleaved per 2-slab k-group so consecutive matmuls share their
    stationary operand (KT slab / Vaug slab); walrus legalization then skips
    the redundant LDWEIGHTS (it dedupes same-stationary runs).
  - Vaug is padded to 128 columns (V^T | ones | zeros) making the PV
    stationary FWL-eligible (fast weight load needs exactly 128 cols) and
    the PV output a full [128, 512] bank; rows 65-127 accumulate zeros.
  - V transposes take both heads at once ([128,128] per 128-key slab).
  - weight/x DMAs are split in halves so the first qkv matmul starts ~1.2us
    after kernel start instead of ~5us.
  - o_proj flushes per 128-token tile (32 small DMAs) to shrink the tail,
    and the out partials are fp16 (halves the out DMA; rel err 9.5e-4).
  - PSUM budget: scores A+B [128,2,512]x2 (4 banks) + PV out [128,512]x2
    (2) + mm scratch [128,512]x2 (2) = 8 banks.
Matmuls run in fp16 (softmax attention is insensitive to score/prob
rounding); the softmax normalizer chain stays fp32r. Emission drains a
fine-grained filler queue (qkv batch 1 / V transposes / o_proj) inside exp
shadows.
"""
import sys

sys.path.insert(0, "/opt/trn_rl_repo")
import numpy as np

NHEADS = 16
HEAD_DIM = 64
HIDDEN = 1024
QKV = NHEADS * HEAD_DIM  # 1024
SCALING = HEAD_DIM ** -0.5
B = 2
S = 2048
T = B * S  # 4096
NCORES = 8
HPC = NHEADS // NCORES  # 2 heads per core
FEAT = HPC * HEAD_DIM  # 128
CHUNK = 512
NCHUNK = S // CHUNK  # 4 per batch
KSLABS = HIDDEN // 128  # 8
SSLABS = S // 128  # 16
NGRP = SSLABS // 2  # 8 k-groups of 2 slabs per (b,h,qc)

_CACHE = {}
LAST_RESULT = None  # BassKernelResults of the most recent kernel() call


def _split_waits(nc, keep=1):
    """Hoist excess per-instruction sem waits into standalone EventSemaphore
    instructions (walrus codegen has small per-opcode wait budgets)."""
    import bass_rust
    import concourse.mybir as mybir

    n_new = 0
    for f in nc.m.functions:
        for blk in f.blocks:
            out = []
            changed = False
            for inst in blk.instructions:
                si = inst.sync_info
                waits = list(si.on_wait) if si is not None else []
                if len(waits) > keep:
                    excess = waits[: len(waits) - keep]
                    kept = waits[len(waits) - keep:]
                    for w in excess:
                        out.append(mybir.InstEventSemaphore(
                            name=f"{inst.name}-esw{n_new}",
                            engine=inst.engine,
                            sync_info=bass_rust.SyncInfo(on_wait=[w], on_update=[]),
                        ))
                        n_new += 1
                    inst.sync_info = bass_rust.SyncInfo(
                        on_wait=kept, on_update=list(si.on_update))
                    changed = True
                out.append(inst)
            if changed:
                blk.instructions = out
    return n_new


def _dedupe_ldweights(nc):
    """Drop an InstLdweights when the PE weight array already holds the same
    stationary operand (same memref/offset/ap/dtype, loaded by the previous
    Ldweights and not clobbered since). tile_legalize emits one Ldweights per
    Matmult unconditionally; consecutive same-stationary matmuls (k-major A/B
    interleave) make half of them redundant. Waits/updates of a dropped
    instruction are preserved via a standalone EventSemaphore."""
    import bass_rust
    import concourse.mybir as mybir

    def wkey(ap):
        try:
            return (str(ap.memref), ap.offset, str(ap.ap), ap.dtype)
        except Exception:
            return None

    n_rm = 0
    for f in nc.m.functions:
        for blk in f.blocks:
            out = []
            loaded = object()  # sentinel: unknown
            for inst in blk.instructions:
                if isinstance(inst, mybir.InstLdweights):
                    key = wkey(inst.ins[0])
                    if (key is not None and key == loaded
                            and not inst.is_transpose and not inst.perf_mode):
                        si = inst.sync_info
                        waits = list(si.on_wait) if si is not None else []
                        upds = list(si.on_update) if si is not None else []
                        if waits or upds:
                            out.append(mybir.InstEventSemaphore(
                                name=f"{inst.name}-lwes",
                                engine=inst.engine,
                                sync_info=bass_rust.SyncInfo(
                                    on_wait=waits, on_update=upds)))
                        n_rm += 1
                        continue
                    loaded = key if not inst.is_transpose else object()
                    out.append(inst)
                elif isinstance(inst, mybir.InstMatmult):
                    if inst.is_transpose:
                        loaded = object()  # transpose streams via weight path
                    out.append(inst)
                elif isinstance(inst, mybir.InstEventSemaphore):
                    out.append(inst)
                elif inst.engine == mybir.EngineType.PE:
                    loaded = object()  # unknown PE inst: be conservative
                    out.append(inst)
                else:
                    out.append(inst)
            blk.instructions = out
    return n_rm


def _build(reps=1):
    import os
    import concourse.bass as bass
    import concourse.mybir as mybir
    import concourse.tile as tile
    from concourse.masks import make_identity

    if os.environ.get("KERNEL_LDW_OPT"):
        _patch_ldw_opt()

    f32 = mybir.dt.float32
    f32r = mybir.dt.float32r
    f16 = mybir.dt.float16
    Exp = mybir.ActivationFunctionType.Exp

    nc = bass.Bass()
    xT = nc.dram_tensor("xT", [HIDDEN, T], f16, kind="ExternalInput")
    wqkvT = nc.dram_tensor("wqkvT", [HIDDEN, 3 * FEAT], f16, kind="ExternalInput")
    bqkv = nc.dram_tensor("bqkv", [FEAT, 3], f32, kind="ExternalInput")
    woT = nc.dram_tensor("woT", [FEAT, HIDDEN], f16, kind="ExternalInput")
    out_d = nc.dram_tensor("out", [T, HIDDEN], f16, kind="ExternalOutput")

    with tile.TileContext(nc) as tc, nc.allow_low_precision(reason="fp16 matmuls"):
        with (
            tc.tile_pool(name="sing", bufs=1) as sing,
            tc.tile_pool(name="xp", bufs=2) as xp,
            tc.tile_pool(name="pp", bufs=4) as pp,
            tc.tile_pool(name="stg", bufs=2) as stg,
            tc.tile_pool(name="sm", bufs=4) as sm,
            tc.tile_pool(name="op", bufs=3) as op,
            tc.tile_pool(name="ps_mm", bufs=2, space="PSUM") as ps_mm,
            tc.tile_pool(name="ps_s", bufs=2, space="PSUM") as ps_s,
            tc.tile_pool(name="ps_o", bufs=2, space="PSUM") as ps_o,
        ):
            wq_sb = sing.tile([128, KSLABS, 3 * FEAT], f16, tag="wq")
            wo_sb = sing.tile([FEAT, HIDDEN], f16, tag="wo")
            bq_sb = sing.tile([FEAT, 3], f32, tag="bq")
            ident = sing.tile([128, 128], f32, tag="id")
            ones1 = sing.tile([1, HEAD_DIM], f32r, tag="on")
            QT = sing.tile([128, T], f16, tag="qt")
            KT = sing.tile([128, T], f16, tag="kt")
            VT = sing.tile([128, T], f32, tag="vt")
            OT = sing.tile([128, T], f16, tag="ot")
            # padded PV stationary: cols 0-63 V^T, col 64 ones, 65-127 zero
            Vaug = sing.tile([128, B, HPC, SSLABS, 128], f16, tag="va")

            wq_c = wqkvT[:].rearrange("(s p) f -> p s f", p=128)
            nc.sync.dma_start(out=wq_sb[:, 0:KSLABS // 2, :],
                              in_=wq_c[:, 0:KSLABS // 2, :])
            nc.sync.dma_start(out=wq_sb[:, KSLABS // 2:KSLABS, :],
                              in_=wq_c[:, KSLABS // 2:KSLABS, :])
            nc.sync.dma_start(out=bq_sb, in_=bqkv[:])
            nc.sync.dma_start(out=wo_sb, in_=woT[:])
            make_identity(nc, ident)
            ones_f = stg.tile([1, HEAD_DIM], f32, tag="onf")
            nc.vector.memset(ones_f, 1.0)
            nc.vector.tensor_copy(ones1, ones_f)
            # Vaug pad init: ones column then zero tail
            vst = stg.tile([128, B * HPC * SSLABS], f32, tag="vst")
            nc.vector.memset(vst, 1.0)
            nc.vector.tensor_copy(Vaug[:, :, :, :, 64:65], vst)
            zst = stg.tile([128, B * HPC * SSLABS, 63], f32, tag="zst")
            nc.vector.memset(zst, 0.0)
            nc.vector.tensor_copy(
                Vaug[:, :, :, :, 65:128],
                zst[:].rearrange("p (b h s) c -> p b h s c", b=B, h=HPC))

            xT_c = xT[:].rearrange("(s p) t -> p s t", p=128)

            from collections import deque
            filler = deque()

            def drain(n=1):
                for _ in range(n):
                    if filler:
                        filler.popleft()()

            def qkv_dma(b, n):
                g = b * NCHUNK + n
                xc = xp.tile([128, KSLABS, CHUNK], f16, tag="xc", name="xc")
                nc.sync.dma_start(
                    out=xc[:, 0:KSLABS // 2, :],
                    in_=xT_c[:, 0:KSLABS // 2, g * CHUNK:(g + 1) * CHUNK])
                nc.sync.dma_start(
                    out=xc[:, KSLABS // 2:KSLABS, :],
                    in_=xT_c[:, KSLABS // 2:KSLABS, g * CHUNK:(g + 1) * CHUNK])
                return xc

            def qkv_feat(b, n, m, xc):
                g = b * NCHUNK + n
                lo, hi = g * CHUNK, (g + 1) * CHUNK
                dest = (QT, KT, VT)[m]
                acc = ps_mm.tile([128, CHUNK], f32, tag="mm", name="acc")
                for s in range(KSLABS):
                    nc.tensor.matmul(
                        acc, wq_sb[:, s, m * FEAT:(m + 1) * FEAT], xc[:, s, :],
                        start=(s == 0), stop=(s == KSLABS - 1))
                nc.vector.tensor_scalar_add(
                    dest[:, lo:hi], acc, bq_sb[:, m:m + 1])

            def qkv_chunk(b, n):
                xc = qkv_dma(b, n)
                for m in range(3):
                    qkv_feat(b, n, m, xc)

            def vtrans_k(b, k):
                # both heads at once: [128,128] transpose of VT slab k
                tp = ps_mm.tile([128, 128], f32, tag="mm", name="tp")
                nc.tensor.transpose(
                    tp, VT[:, b * S + 128 * k: b * S + 128 * (k + 1)], ident)
                nc.vector.tensor_copy(
                    Vaug[:, b, :, k, 0:HEAD_DIM],
                    tp[:].rearrange("p (h d) -> p h d", h=HPC))

            def vtrans(b, n):
                for k in range(4 * n, 4 * n + 4):
                    vtrans_k(b, k)

            def attn_pair(b, h, qp, mid=None, carry=None):
                """Two interleaved q-chunks (2qp, 2qp+1) of (b,h). Each k-slab
                is one [128, 2, 512] psum tile holding the (qA, qB) halves:
                the two scores matmuls (and the two PV matmuls) of a slab
                share their stationary operand and have identical deps.

                Returns a norm() closure (softmax normalization of this
                pair); the caller runs it inside the NEXT pair (carry=) so
                the PE never head-of-line blocks on the reciprocal chain.
                ps_o rotation keeps the psum alive until norm() has read it.
                """
                hs = slice(64 * h, 64 * h + 64)
                qlo = b * S + 2 * qp * CHUNK
                slA = slice(qlo, qlo + CHUNK)
                slB = slice(qlo + CHUNK, qlo + 2 * CHUNK)
                o_psA = ps_o.tile([128, CHUNK], f32, tag="o", name="opsA")
                o_psB = ps_o.tile([128, CHUNK], f32, tag="o", name="opsB")

                def s_slab(k):
                    st = ps_s.tile([128, 2, CHUNK], f32, tag="s", name="st")
                    ksl = slice(b * S + 128 * k, b * S + 128 * (k + 1))
                    nc.tensor.matmul(st[:, 0, :], KT[hs, ksl], QT[hs, slA],
                                     start=True, stop=True)
                    nc.tensor.matmul(st[:, 1, :], KT[hs, ksl], QT[hs, slB],
                                     start=True, stop=True)
                    pt = pp.tile([128, 2, CHUNK], f16, tag="pt", name="pt")
                    nc.scalar.activation(out=pt, in_=st, func=Exp)
                    return pt

                def pv_slab(k, pt):
                    sta = (k == 0)
                    sto = (k == SSLABS - 1)
                    nc.tensor.matmul(o_psA, Vaug[:, b, h, k, :],
                                     pt[:, 0, :], start=sta, stop=sto)
                    nc.tensor.matmul(o_psB, Vaug[:, b, h, k, :],
                                     pt[:, 1, :], start=sta, stop=sto)

                prev = s_slab(0)
                for k in range(1, SSLABS):
                    if k == 1 and carry is not None:
                        carry()  # previous pair's norm: emitted before this
                        # pair's first PV so the ps_o WAR is tracked, late
                        # enough that the PE never waits on the DVE reciprocal
                    if k == SSLABS // 2 and mid is not None:
                        mid()  # emit qkv chunks 2-3 before slabs needing them
                    cur = s_slab(k)
                    drain()  # filler PE work runs in exp shadows
                    pv_slab(k - 1, prev)
                    prev = cur
                drain()
                pv_slab(SSLABS - 1, prev)

                def norm():
                    # phase-major across A/B so DVE and PE pipeline
                    recs, bps, rbs = [], [], []
                    for o_ps in (o_psA, o_psB):
                        rec = sm.tile([1, CHUNK], f32r, tag="rec")
                        nc.vector.reciprocal(
                            rec, o_ps[HEAD_DIM:HEAD_DIM + 1, :])
                        recs.append(rec)
                    for rec in recs:
                        b_ps = ps_mm.tile([HEAD_DIM, CHUNK], f32, tag="mm")
                        nc.tensor.matmul(b_ps, ones1, rec, start=True, stop=True)
                        bps.append(b_ps)
                    for b_ps in bps:
                        rb = sm.tile([HEAD_DIM, CHUNK], f32, tag="rb")
                        nc.vector.tensor_copy(rb, b_ps)
                        rbs.append(rb)
                    for o_ps, qsl, rb in ((o_psA, slA, rbs[0]),
                                          (o_psB, slB, rbs[1])):
                        nc.vector.tensor_mul(OT[hs, qsl], o_ps[0:HEAD_DIM, :], rb)

                return norm

            def oproj_tile_thunks(t):
                # one 128-token tile -> [128, 1024] fp16 staging -> own DMA
                box = {}

                def alloc():
                    box["ost"] = op.tile([128, HIDDEN], f16, tag="ost", name="ost")

                def mm_half(nh):
                    acc = ps_mm.tile([128, CHUNK], f32, tag="mm", name="acc2")
                    nc.tensor.matmul(
                        acc, OT[:, 128 * t:128 * (t + 1)],
                        wo_sb[:, nh * CHUNK:(nh + 1) * CHUNK],
                        start=True, stop=True)
                    nc.vector.tensor_copy(
                        box["ost"][:, nh * CHUNK:(nh + 1) * CHUNK], acc)

                def flush():
                    nc.sync.dma_start(
                        out=out_d[128 * t:128 * (t + 1), :], in_=box["ost"])

                return [alloc, lambda: mm_half(0), lambda: mm_half(1), flush]

            # ---- emission: qkv(b0) chunks 0-1 up front, then attention with
            # fine-grained filler (qkv b1 / vtrans / oproj) in exp shadows ----
            for _rep in range(reps):
                for n in range(2):
                    qkv_chunk(0, n)
                    vtrans(0, n)

                def rest_of_b0_qkv():
                    for n2 in range(2, NCHUNK):
                        qkv_chunk(0, n2)
                        vtrans(0, n2)

                for n in range(NCHUNK):
                    xc_box = {}

                    def dma_thunk(b=1, n=n, box=xc_box):
                        box["xc"] = qkv_dma(b, n)

                    filler.append(dma_thunk)
                    for m in range(3):
                        filler.append(
                            lambda n=n, m=m, box=xc_box: qkv_feat(1, n, m, box["xc"]))
                    for k in range(4 * n, 4 * n + 4):
                        filler.append(lambda k=k: vtrans_k(1, k))

                # b0 attention; oproj token-tiles queue as soon as both heads
                # of their q-chunks are done
                for i, (h, qp) in enumerate([(0, 0), (0, 1), (1, 0), (1, 1)]):
                    attn_pair(0, h, qp, mid=rest_of_b0_qkv if i == 0 else None)
                    if h == 1:
                        for t in range(8 * qp, 8 * qp + 8):
                            filler.extend(oproj_tile_thunks(t))
                drain(len(filler))  # anything left over
                for h, qp in [(0, 0), (0, 1), (1, 0), (1, 1)]:
                    attn_pair(1, h, qp)
                    if h == 1:
                        for t in range(16 + 8 * qp, 24 + 8 * qp):
                            filler.extend(oproj_tile_thunks(t))
                drain(len(filler))

    import os
    if os.environ.get("KERNEL_LWDEDUP"):
        _dedupe_ldweights(nc)  # unsafe on HW: PE pairs each MM with the next
        # pending LW (consume-one), so BIR-level removal shifts all pairings
    _split_waits(nc)
    return nc


def _patch_ldw_opt():
    """Enable walrus's redundant-load-weight optimization (safe at codegen
    level, unlike BIR-level LW removal): flip the pinned
    --enable-ldw-opt=false in bir_verify_and_optimise's driver command."""
    from concourse import bass_utils

    if getattr(bass_utils, "_ldw_opt_patched", False):
        return
    orig = bass_utils.run_command

    def run_command(cmd, *a, **kw):
        cmd = ["--enable-ldw-opt=true" if c == "--enable-ldw-opt=false" else c
               for c in cmd]
        return orig(cmd, *a, **kw)

    bass_utils.run_command = run_command
    bass_utils._ldw_opt_patched = True


def make_in_maps(hidden_states, w_qkv, b_qkv, w_o, b_o):
    x16 = np.ascontiguousarray(
        np.asarray(hidden_states, dtype=np.float32).reshape(T, HIDDEN).T
    ).astype(np.float16)
    w_qkv = np.asarray(w_qkv, dtype=np.float32)
    b_qkv = np.asarray(b_qkv, dtype=np.float32)
    w_o = np.asarray(w_o, dtype=np.float32)

    in_maps = []
    for c in range(NCORES):
        rq = slice(c * FEAT, (c + 1) * FEAT)
        wq = w_qkv[0:QKV][rq] * SCALING
        wk = w_qkv[QKV:2 * QKV][rq]
        wv = w_qkv[2 * QKV:3 * QKV][rq]
        bq = b_qkv[0:QKV][rq] * SCALING
        bk = b_qkv[QKV:2 * QKV][rq]
        bv = b_qkv[2 * QKV:3 * QKV][rq]
        in_maps.append({
            "xT": x16,
            "wqkvT": np.ascontiguousarray(
                np.concatenate([wq, wk, wv], axis=0).T).astype(np.float16),
            "bqkv": np.ascontiguousarray(np.stack([bq, bk, bv], axis=1)),
            "woT": np.ascontiguousarray(w_o[:, rq].T).astype(np.float16),
        })
    return in_maps


def kernel(hidden_states, w_qkv, b_qkv, w_o, b_o):
    global LAST_RESULT
    from concourse.bass_utils import run_bass_kernel_spmd
    import os

    if "nc" not in _CACHE:
        _CACHE["nc"] = _build()
    nc = _CACHE["nc"]

    b_o = np.asarray(b_o, dtype=np.float32)
    in_maps = make_in_maps(hidden_states, w_qkv, b_qkv, w_o, b_o)

    trace = bool(os.environ.get("KERNEL_TRACE"))
    res = run_bass_kernel_spmd(nc, in_maps, list(range(NCORES)), trace=trace)
    LAST_RESULT = res

    acc = np.zeros((T, HIDDEN), dtype=np.float64)
    for c in range(NCORES):
        acc += res.results[c]["out"].astype(np.float64)
    out = (acc + b_o).astype(np.float32).reshape(B, S, HIDDEN)
    return out


# revision 35
# speedup vs baseline: 1.0069x; 1.0069x over previous
"""Multi-head attention (B=2, S=2048, H=1024, 16 heads x 64d) on 8 trn2 cores.

Sharding: tensor-parallel over heads (2 heads/core). Each core computes the
qkv projection for its 384 output features, attention for its 2 heads, and a
partial o_proj ([4096,1024] over its 128-feature slice). Host sums the 8
fp16 partials and adds b_o.

v2 layout/scheduling notes (vs the 306us baseline):
  - attention runs as (b, h, qc-pair) passes: two 512-col q-chunks (A/B)
    interleaved per 2-slab k-group so consecutive matmuls share their
    stationary operand (KT slab / Vaug slab); walrus legalization then skips
    the redundant LDWEIGHTS (it dedupes same-stationary runs).
  - Vaug is padded to 128 columns (V^T | ones | zeros) making the PV
    stationary FWL-eligible (fast weight load needs exactly 128 cols) and
    the PV output a full [128, 512] bank; rows 65-127 accumulate zeros.
  - V transposes take both heads at once ([128,128] per 128-key slab).
  - weight/x DMAs are split in halves so the first qkv matmul starts ~1.2us
    after kernel start instead of ~5us.
  - o_proj flushes per 128-token tile (32 small DMAs) to shrink the tail,
    and the out partials are fp16 (halves the out DMA; rel err 9.5e-4).
  - PSUM budget: scores A+B [128,2,512]x2 (4 banks) + PV out [128,512]x2
    (2) + mm scratch [128,512]x2 (2) = 8 banks.
Matmuls run in fp16 (softmax attention is insensitive to score/prob
rounding); the softmax normalizer chain stays fp32r. Emission drains a
fine-grained filler queue (qkv batch 1 / V transposes / o_proj) inside exp
shadows.
"""
import sys

sys.path.insert(0, "/opt/trn_rl_repo")
import numpy as np

NHEADS = 16
HEAD_DIM = 64
HIDDEN = 1024
QKV = NHEADS * HEAD_DIM  # 1024
SCALING = HEAD_DIM ** -0.5
B = 2
S = 2048
T = B * S  # 4096
NCORES = 8
HPC = NHEADS // NCORES  # 2 heads per core
FEAT = HPC * HEAD_DIM  # 128
CHUNK = 512
NCHUNK = S // CHUNK  # 4 per batch
KSLABS = HIDDEN // 128  # 8
SSLABS = S // 128  # 16
NGRP = SSLABS // 2  # 8 k-groups of 2 slabs per (b,h,qc)

_CACHE = {}
LAST_RESULT = None  # BassKernelResults of the most recent kernel() call


def _split_waits(nc, keep=1):
    """Hoist excess per-instruction sem waits into standalone EventSemaphore
    instructions (walrus codegen has small per-opcode wait budgets)."""
    import bass_rust
    import concourse.mybir as mybir

    n_new = 0
    for f in nc.m.functions:
        for blk in f.blocks:
            out = []
            changed = False
            for inst in blk.instructions:
                si = inst.sync_info
                waits = list(si.on_wait) if si is not None else []
                if len(waits) > keep:
                    excess = waits[: len(waits) - keep]
                    kept = waits[len(waits) - keep:]
                    for w in excess:
                        out.append(mybir.InstEventSemaphore(
                            name=f"{inst.name}-esw{n_new}",
                            engine=inst.engine,
                            sync_info=bass_rust.SyncInfo(on_wait=[w], on_update=[]),
                        ))
                        n_new += 1
                    inst.sync_info = bass_rust.SyncInfo(
                        on_wait=kept, on_update=list(si.on_update))
                    changed = True
                out.append(inst)
            if changed:
                blk.instructions = out
    return n_new


def _dedupe_ldweights(nc):
    """Drop an InstLdweights when the PE weight array already holds the same
    stationary operand (same memref/offset/ap/dtype, loaded by the previous
    Ldweights and not clobbered since). tile_legalize emits one Ldweights per
    Matmult unconditionally; consecutive same-stationary matmuls (k-major A/B
    interleave) make half of them redundant. Waits/updates of a dropped
    instruction are preserved via a standalone EventSemaphore."""
    import bass_rust
    import concourse.mybir as mybir

    def wkey(ap):
        try:
            return (str(ap.memref), ap.offset, str(ap.ap), ap.dtype)
        except Exception:
            return None

    n_rm = 0
    for f in nc.m.functions:
        for blk in f.blocks:
            out = []
            loaded = object()  # sentinel: unknown
            for inst in blk.instructions:
                if isinstance(inst, mybir.InstLdweights):
                    key = wkey(inst.ins[0])
                    if (key is not None and key == loaded
                            and not inst.is_transpose and not inst.perf_mode):
                        si = inst.sync_info
                        waits = list(si.on_wait) if si is not None else []
                        upds = list(si.on_update) if si is not None else []
                        if waits or upds:
                            out.append(mybir.InstEventSemaphore(
                                name=f"{inst.name}-lwes",
                                engine=inst.engine,
                                sync_info=bass_rust.SyncInfo(
                                    on_wait=waits, on_update=upds)))
                        n_rm += 1
                        continue
                    loaded = key if not inst.is_transpose else object()
                    out.append(inst)
                elif isinstance(inst, mybir.InstMatmult):
                    if inst.is_transpose:
                        loaded = object()  # transpose streams via weight path
                    out.append(inst)
                elif isinstance(inst, mybir.InstEventSemaphore):
                    out.append(inst)
                elif inst.engine == mybir.EngineType.PE:
                    loaded = object()  # unknown PE inst: be conservative
                    out.append(inst)
                else:
                    out.append(inst)
            blk.instructions = out
    return n_rm


def _build(reps=1):
    import os
    import concourse.bass as bass
    import concourse.mybir as mybir
    import concourse.tile as tile
    from concourse.masks import make_identity

    if os.environ.get("KERNEL_LDW_OPT"):
        _patch_ldw_opt()

    f32 = mybir.dt.float32
    f32r = mybir.dt.float32r
    f16 = mybir.dt.float16
    Exp = mybir.ActivationFunctionType.Exp

    nc = bass.Bass()
    xT = nc.dram_tensor("xT", [HIDDEN, T], f16, kind="ExternalInput")
    wqkvT = nc.dram_tensor("wqkvT", [HIDDEN, 3 * FEAT], f16, kind="ExternalInput")
    bqkv = nc.dram_tensor("bqkv", [FEAT, 3], f32, kind="ExternalInput")
    woT = nc.dram_tensor("woT", [FEAT, HIDDEN], f16, kind="ExternalInput")
    out_d = nc.dram_tensor("out", [T, HIDDEN], f16, kind="ExternalOutput")

    with tile.TileContext(nc) as tc, nc.allow_low_precision(reason="fp16 matmuls"):
        with (
            tc.tile_pool(name="sing", bufs=1) as sing,
            tc.tile_pool(name="xp", bufs=2) as xp,
            tc.tile_pool(name="pp", bufs=4) as pp,
            tc.tile_pool(name="stg", bufs=2) as stg,
            tc.tile_pool(name="sm", bufs=4) as sm,
            tc.tile_pool(name="op", bufs=6) as op,
            tc.tile_pool(name="ps_mm", bufs=2, space="PSUM") as ps_mm,
            tc.tile_pool(name="ps_s", bufs=2, space="PSUM") as ps_s,
            tc.tile_pool(name="ps_o", bufs=2, space="PSUM") as ps_o,
        ):
            wq_sb = sing.tile([128, KSLABS, 3 * FEAT], f16, tag="wq")
            wo_sb = sing.tile([FEAT, HIDDEN], f16, tag="wo")
            bq_sb = sing.tile([FEAT, 3], f32, tag="bq")
            ident = sing.tile([128, 128], f16, tag="id")
            ones1 = sing.tile([1, HEAD_DIM], f32r, tag="on")
            QT = sing.tile([128, T], f16, tag="qt")
            KT = sing.tile([128, T], f16, tag="kt")
            VT = sing.tile([128, T], f16, tag="vt")
            OT = sing.tile([128, T], f16, tag="ot")
            # padded PV stationary: cols 0-63 V^T, col 64 ones, 65-127 zero
            Vaug = sing.tile([128, B, HPC, SSLABS, 128], f16, tag="va")

            # startup order (DMAs execute roughly in emission order): the
            # q-column slice of the weights first (0.8us), then x chunk 0,
            # then the k/v weight columns and wo — the first qkv matmul
            # starts ~2.5us in instead of ~8us
            wq_c = wqkvT[:].rearrange("(s p) f -> p s f", p=128)
            nc.sync.dma_start(out=wq_sb[:, :, 0:FEAT], in_=wq_c[:, :, 0:FEAT])
            nc.sync.dma_start(out=bq_sb, in_=bqkv[:])
            make_identity(nc, ident)
            ones_f = stg.tile([1, HEAD_DIM], f32, tag="onf")
            nc.vector.memset(ones_f, 1.0)
            nc.vector.tensor_copy(ones1, ones_f)
            # Vaug pad init: ones column then zero tail
            vst = stg.tile([128, B * HPC * SSLABS], f32, tag="vst")
            nc.vector.memset(vst, 1.0)
            nc.vector.tensor_copy(Vaug[:, :, :, :, 64:65], vst)
            zst = stg.tile([128, B * HPC * SSLABS, 63], f32, tag="zst")
            nc.vector.memset(zst, 0.0)
            nc.vector.tensor_copy(
                Vaug[:, :, :, :, 65:128],
                zst[:].rearrange("p (b h s) c -> p b h s c", b=B, h=HPC))

            xT_c = xT[:].rearrange("(s p) t -> p s t", p=128)

            from collections import deque
            # (pe_cost_ns, thunk) queues: req must complete before b1
            # attention (b1 qkv/vtrans); opt is o_proj, drained for PE
            # filler wherever attention leaves bubbles
            filler_req = deque()
            filler_opt = deque()

            def drain(budget=250):
                # pop ~budget ns of PE work (required queue first)
                acc = 0
                while acc < budget and (filler_req or filler_opt):
                    q = filler_req if filler_req else filler_opt
                    cost, thunk = q.popleft()
                    thunk()
                    acc += max(cost, 1)

            def drain_all(q):
                while q:
                    q.popleft()[1]()

            def qkv_dma(b, n):
                g = b * NCHUNK + n
                xc = xp.tile([128, KSLABS, CHUNK], f16, tag="xc", name="xc")
                nc.sync.dma_start(
                    out=xc[:, 0:KSLABS // 2, :],
                    in_=xT_c[:, 0:KSLABS // 2, g * CHUNK:(g + 1) * CHUNK])
                nc.sync.dma_start(
                    out=xc[:, KSLABS // 2:KSLABS, :],
                    in_=xT_c[:, KSLABS // 2:KSLABS, g * CHUNK:(g + 1) * CHUNK])
                return xc

            def qkv_feat(b, n, m, xc):
                g = b * NCHUNK + n
                lo, hi = g * CHUNK, (g + 1) * CHUNK
                dest = (QT, KT, VT)[m]
                acc = ps_mm.tile([128, CHUNK], f32, tag="mm", name="acc")
                for s in range(KSLABS):
                    nc.tensor.matmul(
                        acc, wq_sb[:, s, m * FEAT:(m + 1) * FEAT], xc[:, s, :],
                        start=(s == 0), stop=(s == KSLABS - 1))
                nc.vector.tensor_scalar_add(
                    dest[:, lo:hi], acc, bq_sb[:, m:m + 1])

            def qkv_chunk(b, n):
                xc = qkv_dma(b, n)
                for m in range(3):
                    qkv_feat(b, n, m, xc)

            def vtrans_k(b, k):
                # both heads at once: [128,128] fp16 transpose of VT slab k
                # (fp16 moving identity -> 1 cycle/row and FWL weight load)
                tp = ps_mm.tile([128, 128], f16, tag="mm", name="tp")
                nc.tensor.transpose(
                    tp, VT[:, b * S + 128 * k: b * S + 128 * (k + 1)], ident)
                nc.vector.tensor_copy(
                    Vaug[:, b, :, k, 0:HEAD_DIM],
                    tp[:].rearrange("p (h d) -> p h d", h=HPC))

            def vtrans(b, n):
                for k in range(4 * n, 4 * n + 4):
                    vtrans_k(b, k)

            def attn_pair(b, h, qp, mid=None, carry=None, budget=250):
                """Two interleaved q-chunks (2qp, 2qp+1) of (b,h). Each k-slab
                is one [128, 2, 512] psum tile holding the (qA, qB) halves:
                the two scores matmuls (and the two PV matmuls) of a slab
                share their stationary operand and have identical deps.

                Returns a norm() closure (softmax normalization of this
                pair); the caller runs it inside the NEXT pair (carry=) so
                the PE never head-of-line blocks on the reciprocal chain.
                ps_o rotation keeps the psum alive until norm() has read it.
                """
                hs = slice(64 * h, 64 * h + 64)
                qlo = b * S + 2 * qp * CHUNK
                slA = slice(qlo, qlo + CHUNK)
                slB = slice(qlo + CHUNK, qlo + 2 * CHUNK)
                o_psA = ps_o.tile([128, CHUNK], f32, tag="o", name="opsA")
                o_psB = ps_o.tile([128, CHUNK], f32, tag="o", name="opsB")

                def s_slab(k):
                    st = ps_s.tile([128, 2, CHUNK], f32, tag="s", name="st")
                    ksl = slice(b * S + 128 * k, b * S + 128 * (k + 1))
                    nc.tensor.matmul(st[:, 0, :], KT[hs, ksl], QT[hs, slA],
                                     start=True, stop=True)
                    nc.tensor.matmul(st[:, 1, :], KT[hs, ksl], QT[hs, slB],
                                     start=True, stop=True)
                    pt = pp.tile([128, 2, CHUNK], f16, tag="pt", name="pt")
                    nc.scalar.activation(out=pt, in_=st, func=Exp)
                    return pt

                def pv_slab(k, pt):
                    sta = (k == 0)
                    sto = (k == SSLABS - 1)
                    nc.tensor.matmul(o_psA, Vaug[:, b, h, k, :],
                                     pt[:, 0, :], start=sta, stop=sto)
                    nc.tensor.matmul(o_psB, Vaug[:, b, h, k, :],
                                     pt[:, 1, :], start=sta, stop=sto)

                # PV runs at lag 2 behind scores: exp(k-2) completed well
                # before, so the PE never waits on the ACT->sem chain.
                pts = [s_slab(0)]
                for k in range(1, SSLABS):
                    if k == 1 and carry is not None:
                        carry()  # previous pair's norm: emitted before this
                        # pair's first PV so the ps_o WAR is tracked, late
                        # enough that the PE never waits on the DVE reciprocal
                    if k == SSLABS // 2 and mid is not None:
                        mid()  # emit qkv chunks 2-3 before slabs needing them
                    pts.append(s_slab(k))
                    drain(budget)  # filler PE work runs in exp shadows
                    if k >= 2:
                        pv_slab(k - 2, pts[k - 2])
                drain(budget)
                pv_slab(SSLABS - 2, pts[SSLABS - 2])
                pv_slab(SSLABS - 1, pts[SSLABS - 1])

                def norm():
                    # phase-major across A/B so DVE and PE pipeline
                    recs, bps, rbs = [], [], []
                    for o_ps in (o_psA, o_psB):
                        rec = sm.tile([1, CHUNK], f32r, tag="rec")
                        nc.vector.reciprocal(
                            rec, o_ps[HEAD_DIM:HEAD_DIM + 1, :])
                        recs.append(rec)
                    for rec in recs:
                        b_ps = ps_mm.tile([HEAD_DIM, CHUNK], f32, tag="mm")
                        nc.tensor.matmul(b_ps, ones1, rec, start=True, stop=True)
                        bps.append(b_ps)
                    for b_ps in bps:
                        rb = sm.tile([HEAD_DIM, CHUNK], f32, tag="rb")
                        nc.vector.tensor_copy(rb, b_ps)
                        rbs.append(rb)
                    for o_ps, qsl, rb in ((o_psA, slA, rbs[0]),
                                          (o_psB, slB, rbs[1])):
                        nc.vector.tensor_mul(OT[hs, qsl], o_ps[0:HEAD_DIM, :], rb)

                return norm

            def oproj_tile_thunks(t, late=False):
                # one 128-token tile -> [128, 1024] fp16 staging -> own DMA.
                # late (post-attention) tiles borrow the idle scores psum pool
                # so four acc buffers pipeline the tail.
                box = {}

                def alloc():
                    box["ost"] = op.tile([128, HIDDEN], f16, tag="ost", name="ost")

                def mm_half(nh):
                    if late and nh == 0:
                        # borrow an idle scores buffer (same tag -> same two
                        # rotating psum slots, no extra PSUM footprint)
                        acc = ps_s.tile([128, CHUNK], f32, tag="s",
                                        name="acc3")
                    else:
                        acc = ps_mm.tile([128, CHUNK], f32, tag="mm",
                                         name="acc2")
                    nc.tensor.matmul(
                        acc, OT[:, 128 * t:128 * (t + 1)],
                        wo_sb[:, nh * CHUNK:(nh + 1) * CHUNK],
                        start=True, stop=True)
                    # GPSIMD cannot access PSUM; these copies must stay on DVE
                    nc.vector.tensor_copy(
                        box["ost"][:, nh * CHUNK:(nh + 1) * CHUNK], acc)

                def flush():
                    # late flushes ride the ACT HWDGE queue (idle by then),
                    # parallel to SP's — halves tail DMA serialization
                    eng = nc.scalar if late else nc.sync
                    eng.dma_start(
                        out=out_d[128 * t:128 * (t + 1), :], in_=box["ost"])

                return [(0, alloc), (210, lambda: mm_half(0)),
                        (210, lambda: mm_half(1)), (0, flush)]

            # ---- emission: qkv(b0) chunks 0-1 up front, then attention with
            # fine-grained filler (qkv b1 / vtrans / oproj) in exp shadows ----
            for _rep in range(reps):
                if _rep == 0:
                    # chunk 0 interleaved with the remaining setup DMAs
                    xc0 = qkv_dma(0, 0)
                    qkv_feat(0, 0, 0, xc0)
                    nc.sync.dma_start(out=wq_sb[:, :, FEAT:2 * FEAT],
                                      in_=wq_c[:, :, FEAT:2 * FEAT])
                    nc.sync.dma_start(out=wq_sb[:, :, 2 * FEAT:3 * FEAT],
                                      in_=wq_c[:, :, 2 * FEAT:3 * FEAT])
                    nc.sync.dma_start(out=wo_sb, in_=woT[:])
                    qkv_feat(0, 0, 1, xc0)
                    qkv_feat(0, 0, 2, xc0)
                    vtrans(0, 0)
                    qkv_chunk(0, 1)
                    vtrans(0, 1)
                else:
                    for n in range(2):
                        qkv_chunk(0, n)
                        vtrans(0, n)

                def rest_of_b0_qkv():
                    for n2 in range(2, NCHUNK):
                        qkv_chunk(0, n2)
                        vtrans(0, n2)

                for n in range(NCHUNK):
                    xc_box = {}

                    def dma_thunk(b=1, n=n, box=xc_box):
                        box["xc"] = qkv_dma(b, n)

                    filler_req.append((0, dma_thunk))
                    for m in range(3):
                        filler_req.append((1700, lambda n=n, m=m, box=xc_box:
                                           qkv_feat(1, n, m, box["xc"])))
                    for k in range(4 * n, 4 * n + 4):
                        filler_req.append((60, lambda k=k: vtrans_k(1, k)))

                # attention; each pair's softmax norm is deferred into the
                # next pair (carry); oproj token-tiles queue as soon as both
                # heads of their q-chunks are done (their OT deps are always
                # emitted before the drains that can reach them)
                carry = None
                for i, (h, qp) in enumerate([(0, 0), (0, 1), (1, 0), (1, 1)]):
                    carry = attn_pair(
                        0, h, qp,
                        mid=rest_of_b0_qkv if i == 0 else None, carry=carry,
                        budget=0 if i == 0 else 450)
                    if h == 1:
                        for t in range(8 * qp, 8 * qp + 8):
                            filler_opt.extend(oproj_tile_thunks(t))
                carry()  # (0,1,qp1) norm, before the required-queue flush
                carry = None
                drain_all(filler_req)  # b1 attention needs all of b1 qkv
                for h, qp in [(0, 0), (0, 1), (1, 0), (1, 1)]:
                    carry = attn_pair(1, h, qp, carry=carry, budget=450)
                    if h == 1 and qp == 0:
                        for t in range(16, 24):
                            filler_opt.extend(oproj_tile_thunks(t))
                carry()  # last pair's norm
                drain_all(filler_opt)
                for t in range(24, 32):  # post-attention tail: pipelined path
                    for _, thunk in oproj_tile_thunks(t, late=True):
                        thunk()

    import os
    if os.environ.get("KERNEL_LWDEDUP"):
        _dedupe_ldweights(nc)  # unsafe on HW: PE pairs each MM with the next
        # pending LW (consume-one), so BIR-level removal shifts all pairings
    _split_waits(nc)
    return nc


def _patch_ldw_opt():
    """Enable walrus's redundant-load-weight optimization (safe at codegen
    level, unlike BIR-level LW removal): flip the pinned
    --enable-ldw-opt=false in bir_verify_and_optimise's driver command."""
    from concourse import bass_utils

    if getattr(bass_utils, "_ldw_opt_patched", False):
        return
    orig = bass_utils.run_command

    def run_command(cmd, *a, **kw):
        cmd = ["--enable-ldw-opt=true" if c == "--enable-ldw-opt=false" else c
               for c in cmd]
        return orig(cmd, *a, **kw)

    bass_utils.run_command = run_command
    bass_utils._ldw_opt_patched = True


def make_in_maps(hidden_states, w_qkv, b_qkv, w_o, b_o):
    x16 = np.ascontiguousarray(
        np.asarray(hidden_states, dtype=np.float32).reshape(T, HIDDEN).T
    ).astype(np.float16)
    w_qkv = np.asarray(w_qkv, dtype=np.float32)
    b_qkv = np.asarray(b_qkv, dtype=np.float32)
    w_o = np.asarray(w_o, dtype=np.float32)

    in_maps = []
    for c in range(NCORES):
        rq = slice(c * FEAT, (c + 1) * FEAT)
        wq = w_qkv[0:QKV][rq] * SCALING
        wk = w_qkv[QKV:2 * QKV][rq]
        wv = w_qkv[2 * QKV:3 * QKV][rq]
        bq = b_qkv[0:QKV][rq] * SCALING
        bk = b_qkv[QKV:2 * QKV][rq]
        bv = b_qkv[2 * QKV:3 * QKV][rq]
        in_maps.append({
            "xT": x16,
            "wqkvT": np.ascontiguousarray(
                np.concatenate([wq, wk, wv], axis=0).T).astype(np.float16),
            "bqkv": np.ascontiguousarray(np.stack([bq, bk, bv], axis=1)),
            "woT": np.ascontiguousarray(w_o[:, rq].T).astype(np.float16),
        })
    return in_maps


def kernel(hidden_states, w_qkv, b_qkv, w_o, b_o):
    global LAST_RESULT
    from concourse.bass_utils import run_bass_kernel_spmd
    import os

    if "nc" not in _CACHE:
        _CACHE["nc"] = _build()
    nc = _CACHE["nc"]

    b_o = np.asarray(b_o, dtype=np.float32)
    in_maps = make_in_maps(hidden_states, w_qkv, b_qkv, w_o, b_o)

    trace = bool(os.environ.get("KERNEL_TRACE"))
    res = run_bass_kernel_spmd(nc, in_maps, list(range(NCORES)), trace=trace)
    LAST_RESULT = res

    acc = np.zeros((T, HIDDEN), dtype=np.float64)
    for c in range(NCORES):
        acc += res.results[c]["out"].astype(np.float64)
    out = (acc + b_o).astype(np.float32).reshape(B, S, HIDDEN)
    return out
